# revision 1
# baseline (speedup 1.0000x reference)
"""Trainium2 Bass kernel for nn_LNon_37460704756094 (embedding_lookup).

Math (reference):
    d   = (data - mean(data)) / std(data, ddof=1) * scalei
    s   = sigmoid(d); t = tanh(d)
    theta = interp(theta_lut, s * 119)   # theta_lut = linspace(-pi, pi, 120)
    velo  = interp(velo_lut, |t| * 119)  # velo_lut  = linspace(0, 3, 120)
    val = d * exp(velo * sin(theta)) + velo * cos(theta)
    out = (val - mean(val)) / std(val, ddof=1) * scaleo

Both LUTs are affine in the index, so linear interpolation collapses to an
affine map of the (continuous) index:
    theta = th0 + (th119 - th0) * s        (exact for an affine LUT)
    velo  = (v119 - v0) * |t|  (+ v0, asserted ~0)
cos(theta) = sin(theta + pi/2), so everything becomes Sigmoid/Tanh/Abs/Sin/
Exp/Square activations + a few vector ops. The affine coefficients are read
from the actual `params` input on the host at call time.

Distribution: batch-sharded over 8 cores (4 batches each = [128, 32768] f32
per core, SBUF-resident). Global mean/std for both normalizations via
per-partition accumulation -> partition_all_reduce -> 8-core AllReduce of a
[128, 2] stats buffer. HBM traffic is one 16 MiB read + one 16 MiB write
per core.
"""

import math

import numpy as np

import concourse.bacc as bacc
import concourse.bass as bass
import concourse.mybir as mybir
import concourse.tile as tile
from concourse.bass_utils import run_bass_kernel_spmd

N_CORES = 8
P = 128
B_FULL, C, H, W = 32, 64, 128, 128
PER_CORE = B_FULL // N_CORES * C * H * W          # 4,194,304
FREE = PER_CORE // P                              # 32,768
F = 1024                                          # tile free size
NT = FREE // F                                    # 32 tiles
N_TOTAL = B_FULL * C * H * W                      # 33,554,432

AF = mybir.ActivationFunctionType
ALU = mybir.AluOpType
AX = mybir.AxisListType
F32 = mybir.dt.float32

LAST_RESULT = None  # BassKernelResults of the most recent run (for test.py)

_KERNEL_CACHE = {}


def _build(consts, sim_mode=False):
    """Build the SPMD Bass program. `consts` = (th0, th_slope, v_slope)."""
    th0, th_slope, v_slope = consts
    halfpi = math.pi / 2.0

    nc = bacc.Bacc(None, num_devices=N_CORES)

    # Register the Sin biases as const APs (activation float biases are
    # looked up in nc.const_aps). Same pattern as Bass.__init__.
    for cv in (th0, th0 + halfpi):
        if (F32, cv) not in nc.const_aps.aps:
            t = nc.alloc_sbuf_tensor(f"const-f32-{cv}", [P, 1], F32)
            nc.gpsimd.memset(t.ap(), cv)
            nc.const_aps.aps[(F32, cv)] = t.ap()
    nc.all_engine_barrier()

    data_in = nc.dram_tensor("data", [P, FREE], F32, kind="ExternalInput")
    scal_in = nc.dram_tensor("scal", [P, 2], F32, kind="ExternalInput")
    out_dram = nc.dram_tensor("out", [P, FREE], F32, kind="ExternalOutput")

    groups = [list(range(N_CORES))]

    with tile.TileContext(nc) as tc:
        with (
            tc.tile_pool(name="big", bufs=1) as bigpool,
            tc.tile_pool(name="scr", bufs=3) as scr,
            tc.tile_pool(name="small", bufs=1) as smallpool,
            tc.tile_pool(name="psum", bufs=1, space="PSUM") as psumpool,
            tc.tile_pool(name="dram", bufs=1, space="DRAM") as dram,
        ):
            bigs = [bigpool.tile([P, F], F32, name=f"big{j}", tag=f"big{j}") for j in range(NT)]
            # per-tile partial stats: cols [0:NT) sum(x), [NT:2NT) sum(x^2),
            # [2NT:3NT) sum(val), [3NT:4NT) sum(val^2)
            statbuf = smallpool.tile([P, 4 * NT], F32, name="statbuf", tag="statbuf")
            # small scalars; phase A uses cols 0..15, phase B cols 16..31
            sm = smallpool.tile([P, 32], F32, name="sm", tag="sm")
            stA = smallpool.tile([P, 2], F32, name="stA", tag="stA")
            stB = smallpool.tile([P, 2], F32, name="stB", tag="stB")
            scal_all = smallpool.tile([P, 2], F32, name="scal_all", tag="scal_all")
            ones = smallpool.tile([P, P], F32, name="ones", tag="ones")
            psumA = psumpool.tile([P, 2], F32, name="psumA", tag="psumA")
            psumB = psumpool.tile([P, 2], F32, name="psumB", tag="psumB")

            cc_a_in = dram.tile([P, 2], F32, name="cc_a_in", tag="cc_a_in")
            cc_a_out = dram.tile([P, 2], F32, name="cc_a_out", tag="cc_a_out")
            cc_b_in = dram.tile([P, 2], F32, name="cc_b_in", tag="cc_b_in")
            cc_b_out = dram.tile([P, 2], F32, name="cc_b_out", tag="cc_b_out")

            # scalei / scaleo come pre-broadcast from the host as [128, 2]
            nc.gpsimd.dma_start(scal_all[:], scal_in[:])
            nc.vector.memset(ones[:], 1.0)

            # ---------------- Phase A: load + input stats ----------------
            for j in range(NT):
                sl = slice(j * F, (j + 1) * F)
                nc.sync.dma_start(bigs[j][:], data_in[:, sl])
                sq = scr.tile([P, F], F32, name="sq", tag="p")
                nc.scalar.activation(
                    sq[:], bigs[j][:], AF.Square,
                    accum_out=statbuf[:, NT + j : NT + j + 1],
                )
                nc.vector.reduce_sum(
                    statbuf[:, j : j + 1], bigs[j][:], axis=AX.X
                )

            nc.vector.reduce_sum(stA[:, 0:1], statbuf[:, 0:NT], axis=AX.X)
            nc.vector.reduce_sum(stA[:, 1:2], statbuf[:, NT : 2 * NT], axis=AX.X)

            # cross-core AllReduce of the [128, 2] per-partition partials
            nc.gpsimd.dma_start(cc_a_in[:], stA[:])
            if sim_mode:
                nc.gpsimd.dma_start(cc_a_out[:], cc_a_in[:])
            else:
                nc.gpsimd.collective_compute(
                    "AllReduce", ALU.add, replica_groups=groups,
                    ins=[cc_a_in.opt()], outs=[cc_a_out.opt()],
                )
            nc.gpsimd.dma_start(stA[:], cc_a_out[:])
            # ones.T @ stA: reduces across partitions AND broadcasts the
            # totals to every partition in one idle-PE matmul
            nc.tensor.matmul(psumA[:], ones[:], stA[:])
            nc.vector.tensor_copy(sm[:, 0:2], psumA[:])

            # a = scalei / std, b = -mean * a   (std unbiased, ddof=1)
            nc.vector.tensor_scalar_mul(sm[:, 2:3], sm[:, 0:1], 1.0 / N_TOTAL)   # mean
            nc.vector.tensor_mul(sm[:, 3:4], sm[:, 0:1], sm[:, 2:3])             # S1*mean
            nc.vector.tensor_sub(sm[:, 4:5], sm[:, 1:2], sm[:, 3:4])
            nc.vector.tensor_scalar_mul(sm[:, 5:6], sm[:, 4:5], 1.0 / (N_TOTAL - 1))
            nc.scalar.activation(sm[:, 6:7], sm[:, 5:6], AF.Sqrt)                # std
            nc.vector.reciprocal(sm[:, 7:8], sm[:, 6:7])                         # 1/std
            nc.vector.tensor_mul(sm[:, 8:9], sm[:, 7:8], scal_all[:, 0:1])      # a
            nc.vector.tensor_mul(sm[:, 9:10], sm[:, 2:3], sm[:, 8:9])
            nc.vector.tensor_scalar_mul(sm[:, 10:11], sm[:, 9:10], -1.0)         # b
            a_ap = sm[:, 8:9]
            b_ap = sm[:, 10:11]

            # ---------------- Phase B: elementwise chain + val stats -----
            for j in range(NT):
                d = bigs[j][:]
                s_ = scr.tile([P, F], F32, name="s", tag="s")
                t_ = scr.tile([P, F], F32, name="t", tag="t")
                u_ = scr.tile([P, F], F32, name="u", tag="u", bufs=2)
                T3 = scr.tile([P, F], F32, name="T3", tag="T3", bufs=2)
                sin_ = scr.tile([P, F], F32, name="sin", tag="sin")
                cos_ = scr.tile([P, F], F32, name="cos", tag="cos")
                p_ = scr.tile([P, F], F32, name="p", tag="p")

                nc.scalar.activation(s_[:], d, AF.Sigmoid, bias=b_ap, scale=a_ap)
                nc.scalar.activation(t_[:], d, AF.Tanh, bias=b_ap, scale=a_ap)
                nc.vector.tensor_scalar(
                    u_[:], d, a_ap, b_ap, op0=ALU.mult, op1=ALU.add
                )
                nc.scalar.activation(T3[:], t_[:], AF.Abs, scale=v_slope)
                nc.scalar.activation(sin_[:], s_[:], AF.Sin, bias=th0, scale=th_slope)
                nc.scalar.activation(
                    cos_[:], s_[:], AF.Sin, bias=th0 + halfpi, scale=th_slope
                )
                nc.vector.tensor_mul(p_[:], T3[:], sin_[:])
                nc.scalar.activation(sin_[:], p_[:], AF.Exp)                 # e
                nc.vector.tensor_mul(cos_[:], T3[:], cos_[:])                # q
                nc.vector.tensor_mul(u_[:], u_[:], sin_[:])                  # r = u*e
                nc.vector.tensor_add(d, u_[:], cos_[:])                      # val
                nc.scalar.activation(
                    t_[:], d, AF.Square,
                    accum_out=statbuf[:, 3 * NT + j : 3 * NT + j + 1],
                )
                nc.vector.reduce_sum(
                    statbuf[:, 2 * NT + j : 2 * NT + j + 1], d, axis=AX.X
                )

            nc.vector.reduce_sum(stB[:, 0:1], statbuf[:, 2 * NT : 3 * NT], axis=AX.X)
            nc.vector.reduce_sum(stB[:, 1:2], statbuf[:, 3 * NT : 4 * NT], axis=AX.X)

            nc.gpsimd.dma_start(cc_b_in[:], stB[:])
            if sim_mode:
                nc.gpsimd.dma_start(cc_b_out[:], cc_b_in[:])
            else:
                nc.gpsimd.collective_compute(
                    "AllReduce", ALU.add, replica_groups=groups,
                    ins=[cc_b_in.opt()], outs=[cc_b_out.opt()],
                )
            nc.gpsimd.dma_start(stB[:], cc_b_out[:])
            nc.tensor.matmul(psumB[:], ones[:], stB[:])
            nc.vector.tensor_copy(sm[:, 16:18], psumB[:])

            nc.vector.tensor_scalar_mul(sm[:, 18:19], sm[:, 16:17], 1.0 / N_TOTAL)
            nc.vector.tensor_mul(sm[:, 19:20], sm[:, 16:17], sm[:, 18:19])
            nc.vector.tensor_sub(sm[:, 20:21], sm[:, 17:18], sm[:, 19:20])
            nc.vector.tensor_scalar_mul(sm[:, 21:22], sm[:, 20:21], 1.0 / (N_TOTAL - 1))
            nc.scalar.activation(sm[:, 22:23], sm[:, 21:22], AF.Sqrt)
            nc.vector.reciprocal(sm[:, 23:24], sm[:, 22:23])
            nc.vector.tensor_mul(sm[:, 24:25], sm[:, 23:24], scal_all[:, 1:2])  # a2
            nc.vector.tensor_mul(sm[:, 25:26], sm[:, 18:19], sm[:, 24:25])
            nc.vector.tensor_scalar_mul(sm[:, 26:27], sm[:, 25:26], -1.0)        # b2
            a2_ap = sm[:, 24:25]
            b2_ap = sm[:, 26:27]

            # ---------------- Phase C: normalize + store -----------------
            for j in range(NT):
                sl = slice(j * F, (j + 1) * F)
                o_ = scr.tile([P, F], F32, name="o", tag="s")
                nc.vector.tensor_scalar(
                    o_[:], bigs[j][:], a2_ap, b2_ap, op0=ALU.mult, op1=ALU.add
                )
                nc.sync.dma_start(out_dram[:, sl], o_[:])

    nc.finalize()
    return nc


def kernel(data, params, scalei, scaleo):
    global LAST_RESULT
    data = np.ascontiguousarray(np.asarray(data, dtype=np.float32))
    params = np.asarray(params, dtype=np.float32)

    # Affine-LUT coefficients from the actual params input.
    th_lut = params[0, 0]
    v_lut = params[1, 0]
    npts = th_lut.shape[0]
    th0 = float(th_lut[0])
    th_slope = float(th_lut[npts - 1]) - th0
    v0 = float(v_lut[0])
    v_slope = float(v_lut[npts - 1]) - v0
    assert abs(v0) < 1e-6, f"velocity LUT must start at 0 (got {v0})"

    consts = (th0, th_slope, v_slope)
    nc = _KERNEL_CACHE.get(consts)
    if nc is None:
        nc = _build(consts)
        _KERNEL_CACHE[consts] = nc

    scal = np.tile(
        np.array(
            [[float(np.asarray(scalei).reshape(-1)[0]),
              float(np.asarray(scaleo).reshape(-1)[0])]],
            dtype=np.float32,
        ),
        (P, 1),
    )

    bpc = B_FULL // N_CORES
    in_maps = []
    for i in range(N_CORES):
        shard = np.ascontiguousarray(
            data[i * bpc : (i + 1) * bpc]
        ).reshape(P, FREE)
        in_maps.append({"data": shard, "scal": scal})

    res = run_bass_kernel_spmd(nc, in_maps, core_ids=list(range(N_CORES)))
    LAST_RESULT = res

    out = np.concatenate(
        [r["out"].reshape(bpc, C, H, W) for r in res.results], axis=0
    )
    return out

